# revision 10
# baseline (speedup 1.0000x reference)
"""Fused causal multi-head attention block on 8 Trainium2 NeuronCores.

Problem (GPT-2 style attention, B=2, S=2048, D=1024, H=16, hd=64):
    qkv = x @ w_attn + b_attn ; split q,k,v ; per-head causal softmax(q k^T / 8) v
    out = attn_out @ w_proj + b_proj

Sharding: data parallel on batch (2) x tensor parallel on heads (4 groups of 4
heads). Core c -> batch c//4, head group c%4. Each core computes a partial
[S, D] output (its heads' slice of w_proj rows); host sums the 4 partials per
batch and adds b_proj.

Per-core kernel layout trick: scores are computed TRANSPOSED (scoresT[key,
query]) so that the softmax denominator falls out of the attn@v matmul by
appending a ones-column to v: [v | 1]^T @ exp(scoresT) yields both the
unnormalized output and the per-query denominator in one accumulation.
"""

import sys

sys.path.insert(0, "/opt/trn_rl_repo")

import numpy as np

import concourse.bass as bass
import concourse.mybir as mybir
import concourse.tile as tile
from concourse import bacc
from concourse.bass_utils import run_bass_kernel_spmd

F32 = mybir.dt.float32
F32R = mybir.dt.float32r
AFT = mybir.ActivationFunctionType

B, S, D, H, HD = 2, 2048, 1024, 16, 64
NCORES = 8
HPC = 4            # heads per core
CH = HPC * HD      # 256 channels per core
VW = HD + 1        # v width incl. ones column
P = 128
KT = D // P        # 8 contraction tiles over D
SQ = 512           # query/N chunk
NSQ = S // SQ      # 4
NST = S // P       # 16 seq tiles
SCALE = 1.0 / np.sqrt(HD)


def emit_kernel(nc, tc, ap):
    """Emit the per-core program. `ap` is a dict of DRAM APs."""
    with (
        tc.tile_pool(name="const", bufs=1) as cp,
        tc.tile_pool(name="xw", bufs=1) as xw,
        tc.tile_pool(name="act", bufs=1) as acts,
        tc.tile_pool(name="ex", bufs=4) as exp_pool,
        tc.tile_pool(name="osb", bufs=2) as osb,
        tc.tile_pool(name="psA", bufs=3, space="PSUM") as psA,
        tc.tile_pool(name="psB", bufs=2, space="PSUM") as psB,
        tc.tile_pool(name="psC", bufs=3, space="PSUM") as psC,
    ):
        # ---- constants ----
        tri = cp.tile([P, P], F32R, name="tri", tag="tri")
        nc.sync.dma_start(tri, ap["tri"].bitcast(F32R))
        ea = cp.tile([1, P], F32R, name="ea", tag="ea")
        nc.sync.dma_start(ea, ap["ea"].bitcast(F32R))
        eb = cp.tile([1, P], F32R, name="eb", tag="eb")
        nc.sync.dma_start(eb, ap["eb"].bitcast(F32R))
        bq = cp.tile([P, 2], F32, name="bq", tag="bq")
        nc.sync.dma_start(bq, ap["bq"])
        bk = cp.tile([P, 2], F32, name="bk", tag="bk")
        nc.sync.dma_start(bk, ap["bk"])
        bv = cp.tile([1, HPC * VW], F32R, name="bv", tag="bv")
        nc.sync.dma_start(bv, ap["bv"].bitcast(F32R))
        ones1 = cp.tile([1, P], F32R, name="ones1", tag="ones1")
        nc.sync.dma_start(ones1, ap["ones1"].bitcast(F32R))
        recips = [cp.tile([1, S], F32R, name=f"rcp{h}", tag=f"rcp{h}")
                  for h in range(HPC)]

        # ---- weight/x loads (k-tile order so QKV can start early) ----
        xts, wq_t, wk_t, wv_t = [], [], [], []
        for k in range(KT):
            xt = xw.tile([P, S], F32R, name=f"xt{k}", tag=f"xt{k}")
            nc.sync.dma_start(xt, ap["xT"][k * P:(k + 1) * P, :].bitcast(F32R))
            xts.append(xt)
            w = xw.tile([P, CH], F32R, name=f"wq{k}", tag=f"wq{k}")
            nc.sync.dma_start(w, ap["wq"][k * P:(k + 1) * P, :].bitcast(F32R))
            wq_t.append(w)
            w = xw.tile([P, CH], F32R, name=f"wk{k}", tag=f"wk{k}")
            nc.sync.dma_start(w, ap["wk"][k * P:(k + 1) * P, :].bitcast(F32R))
            wk_t.append(w)
            w = xw.tile([P, HPC * VW], F32R, name=f"wv{k}", tag=f"wv{k}")
            nc.sync.dma_start(w, ap["wv"][k * P:(k + 1) * P, :].bitcast(F32R))
            wv_t.append(w)
        wp_t = []
        for k in range(2):
            w = xw.tile([P, D], F32R, name=f"wp{k}", tag=f"wp{k}")
            nc.sync.dma_start(w, ap["wp"][k * P:(k + 1) * P, :].bitcast(F32R))
            wp_t.append(w)

        # ---- activations living across phases ----
        qT = [acts.tile([P, S], F32R, name=f"qT{i}", tag=f"qT{i}") for i in range(2)]
        kTt = [acts.tile([P, S], F32R, name=f"kT{i}", tag=f"kT{i}") for i in range(2)]
        vv = acts.tile([P, NST, HPC * VW], F32R, name="vv", tag="vv")
        outT = [acts.tile([P, S], F32R, name=f"oT{i}", tag=f"oT{i}") for i in range(2)]

        # ---- QKV projection ----
        # qT/kT: [chan, seq] = w^T x^T : lhsT = w k-tile, rhs = xT k-tile
        for dst, wt, bias in ((qT, wq_t, bq), (kTt, wk_t, bk)):
            for c in range(NSQ):
                for i in range(2):
                    ps = psA.tile([P, SQ], F32, name="ps", tag="ps")
                    for k in range(KT):
                        nc.tensor.matmul(
                            ps,
                            wt[k][:, i * P:(i + 1) * P].bitcast(F32R),
                            xts[k][:, c * SQ:(c + 1) * SQ].bitcast(F32R),
                            start=(k == 0),
                            stop=(k == KT - 1),
                        )
                    nc.scalar.activation(
                        dst[i][:, c * SQ:(c + 1) * SQ], ps, AFT.Identity,
                        bias=bias[:, i:i + 1],
                    )
        # v (natural layout, with interleaved ones cols): lhsT = xT k-tile
        for st in range(NST):
            ps = psA.tile([P, SQ], F32, name="psv", tag="ps")
            psv = ps[:, 0:HPC * VW]
            for k in range(KT):
                nc.tensor.matmul(
                    psv,
                    xts[k][:, st * P:(st + 1) * P].bitcast(F32R),
                    wv_t[k].bitcast(F32R),
                    start=(k == 0),
                    stop=False,
                )
            # += ones_col(seq) x (bv | interleaved 1.0) : adds v-bias and the
            # ones column in a single K=1 matmul
            nc.tensor.matmul(
                psv, ones1.bitcast(F32R), bv.bitcast(F32R), start=False, stop=True,
            )
            nc.any.tensor_copy(vv[:, st, :], psv)

        # ---- attention + normalization + output projection, chunk-pipelined ----
        for c in range(NSQ):
            for h in range(HPC):
                ih, ro = h // 2, (h % 2) * 64
                nkt = 4 * (c + 1)
                acc = psB.tile([VW, SQ], F32, name="acc", tag="acc")
                for kt in range(nkt):
                    off = kt * P - c * SQ  # >=0 only on diagonal blocks
                    colo = max(0, off)
                    # scoresT[key, query] for this (key-tile, query-chunk)
                    sc = psC.tile([P, SQ], F32, name="sc", tag="sc")
                    nc.tensor.matmul(
                        sc[:, colo:SQ],
                        kTt[ih][ro:ro + 64, kt * P:(kt + 1) * P].bitcast(F32R),
                        qT[ih][ro:ro + 64, c * SQ + colo:(c + 1) * SQ].bitcast(F32R),
                        start=True,
                        stop=True,
                    )
                    ex = exp_pool.tile([P, SQ], F32R, name="ex", tag="ex")
                    nc.scalar.activation(
                        ex[:, colo:SQ], sc[:, colo:SQ], AFT.Exp, scale=SCALE,
                    )
                    if off >= 0:
                        # partial triangle on the diagonal block
                        nc.vector.tensor_mul(
                            ex[:, colo:colo + P], ex[:, colo:colo + P], tri,
                        )
                    nc.tensor.matmul(
                        acc[:, colo:SQ],
                        vv[:, kt, h * VW:(h + 1) * VW].bitcast(F32R),
                        ex[:, colo:SQ].bitcast(F32R),
                        start=(kt == 0),
                        stop=(kt == nkt - 1),
                    )
                nc.any.tensor_copy(
                    outT[ih][ro:ro + 64, c * SQ:(c + 1) * SQ], acc[0:64, :],
                )
                with nc.allow_low_precision(reason="fp32r recip feeds matmul"):
                    nc.vector.reciprocal(
                        recips[h][:, c * SQ:(c + 1) * SQ], acc[64:65, :],
                    )
            # normalize: outT *= broadcast(recip of its head)
            for i in range(2):
                rb = psA.tile([P, SQ], F32, name="ps", tag="ps")
                nc.tensor.matmul(
                    rb, ea, recips[2 * i][:, c * SQ:(c + 1) * SQ],
                    start=True, stop=False,
                )
                nc.tensor.matmul(
                    rb, eb, recips[2 * i + 1][:, c * SQ:(c + 1) * SQ],
                    start=False, stop=True,
                )
                nc.vector.tensor_mul(
                    outT[i][:, c * SQ:(c + 1) * SQ],
                    outT[i][:, c * SQ:(c + 1) * SQ],
                    rb,
                )
            # partial output projection for this chunk's seq tiles
            for m in range(4 * c, 4 * c + 4):
                for nch in range(2):
                    ps = psA.tile([P, SQ], F32, name="ps", tag="ps")
                    for kk in range(2):
                        nc.tensor.matmul(
                            ps,
                            outT[kk][:, m * P:(m + 1) * P].bitcast(F32R),
                            wp_t[kk][:, nch * SQ:(nch + 1) * SQ].bitcast(F32R),
                            start=(kk == 0),
                            stop=(kk == 1),
                        )
                    ob = osb.tile([P, SQ], F32, name="ob", tag="ob")
                    nc.any.tensor_copy(ob, ps)
                    nc.sync.dma_start(
                        ap["out"][m * P:(m + 1) * P, nch * SQ:(nch + 1) * SQ], ob,
                    )


def build_program():
    nc = bacc.Bacc("TRN2", target_bir_lowering=False, debug=False,
                   num_devices=NCORES)
    ap = {}
    for name, shape in (
        ("xT", [D, S]), ("wq", [D, CH]), ("wk", [D, CH]),
        ("wv", [D, HPC * VW]), ("bq", [P, 2]), ("bk", [P, 2]),
        ("bv", [1, HPC * VW]), ("wp", [CH, D]),
        ("tri", [P, P]), ("ea", [1, P]), ("eb", [1, P]), ("ones1", [1, P]),
    ):
        ap[name] = nc.dram_tensor(name, shape, F32, kind="ExternalInput").ap()
    ap["out"] = nc.dram_tensor("out", [S, D], F32, kind="ExternalOutput").ap()

    with tile.TileContext(nc) as tc:
        emit_kernel(nc, tc, ap)
    nc.compile()
    return nc


def make_core_inputs(hidden_states, w_attn, b_attn, w_proj):
    """Host-side sharding: per-core input dicts (core = batch*4 + head_group)."""
    f32 = np.float32
    x = np.asarray(hidden_states, f32)
    w_attn = np.asarray(w_attn, f32)
    b_attn = np.asarray(b_attn, f32)
    w_proj = np.asarray(w_proj, f32)

    tri = (np.arange(P)[:, None] <= np.arange(P)[None, :]).astype(f32)
    ea = np.zeros((1, P), f32); ea[0, :HD] = 1.0
    eb = np.zeros((1, P), f32); eb[0, HD:] = 1.0
    ones_row = np.ones((1, P), f32)
    xTs = [np.ascontiguousarray(x[b].T) for b in range(B)]

    in_maps = []
    for core in range(NCORES):
        b, g = core // HPC, core % HPC
        wq = np.ascontiguousarray(w_attn[:, g * CH:(g + 1) * CH])
        wk = np.ascontiguousarray(w_attn[:, D + g * CH:D + (g + 1) * CH])
        wv = np.zeros((D, HPC * VW), f32)
        bv = np.zeros((1, HPC * VW), f32)
        for h in range(HPC):
            src = 2 * D + (g * HPC + h) * HD
            wv[:, h * VW:h * VW + HD] = w_attn[:, src:src + HD]
            bv[0, h * VW:h * VW + HD] = b_attn[src:src + HD]
            bv[0, h * VW + HD] = 1.0
        bq = np.ascontiguousarray(
            b_attn[g * CH:(g + 1) * CH].reshape(2, P).T)
        bk = np.ascontiguousarray(
            b_attn[D + g * CH:D + (g + 1) * CH].reshape(2, P).T)
        wp = np.ascontiguousarray(w_proj[g * CH:(g + 1) * CH, :])
        in_maps.append({
            "xT": xTs[b], "wq": wq, "wk": wk, "wv": wv,
            "bq": bq, "bk": bk, "bv": bv, "wp": wp, "tri": tri,
            "ea": ea, "eb": eb, "ones1": ones_row,
        })
    return in_maps


_PROGRAM = None


def kernel(hidden_states, w_attn, b_attn, w_proj, b_proj):
    global _PROGRAM
    if _PROGRAM is None:
        _PROGRAM = build_program()
    in_maps = make_core_inputs(hidden_states, w_attn, b_attn, w_proj)
    res = run_bass_kernel_spmd(_PROGRAM, in_maps, core_ids=list(range(NCORES)))
    out = np.zeros((B, S, D), np.float32)
    for core in range(NCORES):
        out[core // HPC] += res.results[core]["out"]
    out += np.asarray(b_proj, np.float32)
    return out


# revision 11
# speedup vs baseline: 1.3527x; 1.3527x over previous
"""Fused causal multi-head attention block on 8 Trainium2 NeuronCores.

Problem (GPT-2 style attention, B=2, S=2048, D=1024, H=16, hd=64):
    qkv = x @ w_attn + b_attn ; split q,k,v ; per-head causal softmax(q k^T / 8) v
    out = attn_out @ w_proj + b_proj

Sharding: data parallel on batch (2) x tensor parallel on heads (4 groups of 4
heads). Core c -> batch c//4, head group c%4. Each core computes a partial
[S, D] output (its heads' slice of w_proj rows); host sums the 4 partials per
batch and adds b_proj.

Per-core kernel layout tricks:
- scores are computed TRANSPOSED (scoresT[key, query]) so the softmax
  denominator falls out of the attn@v matmul by appending a ones-column to v:
  [v | 1]^T @ exp(scoresT) yields the unnormalized output and the per-query
  denominator in one PSUM accumulation.
- matmul inputs are fp16 (full PE rate + fast weight loads); all
  accumulation is fp32 in PSUM. exp(s/8) is in [0, ~13] and all activations
  are O(1), comfortably inside fp16 range.
- causal masking is free for fully-masked blocks (restricted matmul widths)
  and a single precomputed [128,128] triangle mask handles diagonal blocks.
"""

import sys

sys.path.insert(0, "/opt/trn_rl_repo")

import numpy as np

import concourse.bass as bass
import concourse.mybir as mybir
import concourse.tile as tile
from concourse import bacc
from concourse.bass_utils import run_bass_kernel_spmd

F32 = mybir.dt.float32
F16 = mybir.dt.float16
AFT = mybir.ActivationFunctionType

B, S, D, H, HD = 2, 2048, 1024, 16, 64
NCORES = 8
HPC = 4            # heads per core
CH = HPC * HD      # 256 channels per core
VW = HD + 1        # v width incl. ones column
P = 128
KT = D // P        # 8 contraction tiles over D
SQ = 512           # query/N chunk
NSQ = S // SQ      # 4
NST = S // P       # 16 seq tiles
SCALE = 1.0 / np.sqrt(HD)


def emit_kernel(nc, tc, ap):
    """Emit the per-core program. `ap` is a dict of DRAM APs."""
    with (
        tc.tile_pool(name="const", bufs=1) as cp,
        tc.tile_pool(name="xw", bufs=1) as xw,
        tc.tile_pool(name="act", bufs=1) as acts,
        tc.tile_pool(name="ex", bufs=6) as exp_pool,
        tc.tile_pool(name="dh", bufs=4) as dh_pool,
        tc.tile_pool(name="rc", bufs=3) as rc_pool,
        tc.tile_pool(name="osb", bufs=3) as osb,
        tc.tile_pool(name="psA", bufs=3, space="PSUM") as psA,
        tc.tile_pool(name="psB", bufs=2, space="PSUM") as psB,
        tc.tile_pool(name="psC", bufs=3, space="PSUM") as psC,
    ):
        # ---- constants ----
        tri = cp.tile([P, P], F16, name="tri", tag="tri")
        nc.sync.dma_start(tri, ap["tri"])
        bq = cp.tile([P, 2], F32, name="bq", tag="bq")
        nc.sync.dma_start(bq, ap["bq"])
        bk = cp.tile([P, 2], F32, name="bk", tag="bk")
        nc.sync.dma_start(bk, ap["bk"])
        bv = cp.tile([1, HPC * VW], F16, name="bv", tag="bv")
        nc.sync.dma_start(bv, ap["bv"])
        ones1 = cp.tile([1, P], F16, name="ones1", tag="ones1")
        nc.sync.dma_start(ones1, ap["ones1"])

        # ---- weight/x loads (k-tile order so QKV can start early) ----
        xts, wq_t, wk_t, wv_t = [], [], [], []
        for k in range(KT):
            xt = xw.tile([P, S], F16, name=f"xt{k}", tag=f"xt{k}")
            nc.sync.dma_start(xt, ap["xT"][k * P:(k + 1) * P, :])
            xts.append(xt)
            w = xw.tile([P, CH], F16, name=f"wq{k}", tag=f"wq{k}")
            nc.sync.dma_start(w, ap["wq"][k * P:(k + 1) * P, :])
            wq_t.append(w)
            w = xw.tile([P, CH], F16, name=f"wk{k}", tag=f"wk{k}")
            nc.sync.dma_start(w, ap["wk"][k * P:(k + 1) * P, :])
            wk_t.append(w)
            w = xw.tile([P, HPC * VW], F16, name=f"wv{k}", tag=f"wv{k}")
            nc.sync.dma_start(w, ap["wv"][k * P:(k + 1) * P, :])
            wv_t.append(w)
        wp_t = []
        for k in range(2):
            w = xw.tile([P, D], F16, name=f"wp{k}", tag=f"wp{k}")
            nc.sync.dma_start(w, ap["wp"][k * P:(k + 1) * P, :])
            wp_t.append(w)

        # ---- activations living across phases ----
        qT = [acts.tile([P, S], F16, name=f"qT{i}", tag=f"qT{i}") for i in range(2)]
        kTt = [acts.tile([P, S], F16, name=f"kT{i}", tag=f"kT{i}") for i in range(2)]
        vv = acts.tile([P, NST, HPC * VW], F16, name="vv", tag="vv")
        outT = [acts.tile([P, S], F16, name=f"oT{i}", tag=f"oT{i}") for i in range(2)]

        # ---- QKV projection ----
        # qT/kT: [chan, seq] = w^T x^T : lhsT = w k-tile, rhs = xT k-tile
        for dst, wt, bias in ((qT, wq_t, bq), (kTt, wk_t, bk)):
            for c in range(NSQ):
                for i in range(2):
                    ps = psA.tile([P, SQ], F32, name="ps", tag="ps")
                    for k in range(KT):
                        nc.tensor.matmul(
                            ps,
                            wt[k][:, i * P:(i + 1) * P],
                            xts[k][:, c * SQ:(c + 1) * SQ],
                            start=(k == 0),
                            stop=(k == KT - 1),
                        )
                    # eviction with per-partition bias add (b is zeros in the
                    # reference setup but kept for correctness)
                    with nc.allow_low_precision(reason="fp16 matmul inputs"):
                        nc.vector.tensor_scalar_add(
                            dst[i][:, c * SQ:(c + 1) * SQ], ps, bias[:, i:i + 1],
                        )
        # v (natural layout, with interleaved ones cols): lhsT = xT k-tile
        for st in range(NST):
            ps = psA.tile([P, SQ], F32, name="psv", tag="ps")
            psv = ps[:, 0:HPC * VW]
            for k in range(KT):
                nc.tensor.matmul(
                    psv,
                    xts[k][:, st * P:(st + 1) * P],
                    wv_t[k],
                    start=(k == 0),
                    stop=False,
                )
            # += ones_col(seq) x (bv | interleaved 1.0) : adds v-bias and the
            # ones column in a single K=1 matmul
            nc.tensor.matmul(
                psv, ones1, bv, start=False, stop=True,
            )
            with nc.allow_low_precision(reason="fp16 matmul inputs"):
                nc.vector.tensor_copy(vv[:, st, :], psv)

        # ---- attention + normalization + output projection, chunk-pipelined ----
        for c in range(NSQ):
            denh = []
            for h in range(HPC):
                ih, ro = h // 2, (h % 2) * 64
                nkt = 4 * (c + 1)
                acc = psB.tile([VW, SQ], F32, name="acc", tag="acc")
                for kt in range(nkt):
                    off = kt * P - c * SQ  # >=0 only on diagonal blocks
                    colo = max(0, off)
                    # scoresT[key, query] for this (key-tile, query-chunk)
                    sc = psC.tile([P, SQ], F32, name="sc", tag="sc")
                    nc.tensor.matmul(
                        sc[:, colo:SQ],
                        kTt[ih][ro:ro + 64, kt * P:(kt + 1) * P],
                        qT[ih][ro:ro + 64, c * SQ + colo:(c + 1) * SQ],
                        start=True,
                        stop=True,
                    )
                    ex = exp_pool.tile([P, SQ], F16, name="ex", tag="ex")
                    nc.scalar.activation(
                        ex[:, colo:SQ], sc[:, colo:SQ], AFT.Exp, scale=SCALE,
                    )
                    if off >= 0:
                        # partial triangle on the diagonal block
                        nc.vector.tensor_mul(
                            ex[:, colo:colo + P], ex[:, colo:colo + P], tri,
                        )
                    nc.tensor.matmul(
                        acc[:, colo:SQ],
                        vv[:, kt, h * VW:(h + 1) * VW],
                        ex[:, colo:SQ],
                        start=(kt == 0),
                        stop=(kt == nkt - 1),
                    )
                with nc.allow_low_precision(reason="fp16 matmul inputs"):
                    nc.vector.tensor_copy(
                        outT[ih][ro:ro + 64, c * SQ:(c + 1) * SQ], acc[0:64, :],
                    )
                # denominator row -> fp16 SBUF (feeds the broadcast matmul)
                dn = dh_pool.tile([1, SQ], F16, name="dn", tag="dn")
                nc.scalar.activation(dn, acc[64:65, :], AFT.Copy)
                denh.append(dn)
            # normalize: outT *= 1/denominator (broadcast via K=1 matmul, then
            # one full-width reciprocal on 128 lanes)
            for i in range(2):
                db = psA.tile([P, SQ], F32, name="ps", tag="ps")
                nc.tensor.matmul(
                    db[0:64, :], ones1[:, 0:64], denh[2 * i],
                    start=True, stop=True,
                )
                nc.tensor.matmul(
                    db[64:P, :], ones1[:, 0:64], denh[2 * i + 1],
                    start=True, stop=True,
                )
                rcpb = rc_pool.tile([P, SQ], F16, name="rcpb", tag="rcpb")
                with nc.allow_low_precision(reason="fp16 matmul inputs"):
                    nc.vector.reciprocal(rcpb, db)
                nc.vector.tensor_mul(
                    outT[i][:, c * SQ:(c + 1) * SQ],
                    outT[i][:, c * SQ:(c + 1) * SQ],
                    rcpb,
                )
            # partial output projection for this chunk's seq tiles
            for m in range(4 * c, 4 * c + 4):
                for nch in range(2):
                    ps = psA.tile([P, SQ], F32, name="ps", tag="ps")
                    for kk in range(2):
                        nc.tensor.matmul(
                            ps,
                            outT[kk][:, m * P:(m + 1) * P],
                            wp_t[kk][:, nch * SQ:(nch + 1) * SQ],
                            start=(kk == 0),
                            stop=(kk == 1),
                        )
                    ob = osb.tile([P, SQ], F32, name="ob", tag="ob")
                    nc.vector.tensor_copy(ob, ps)
                    nc.sync.dma_start(
                        ap["out"][m * P:(m + 1) * P, nch * SQ:(nch + 1) * SQ], ob,
                    )


def build_program():
    nc = bacc.Bacc("TRN2", target_bir_lowering=False, debug=False,
                   num_devices=NCORES)
    ap = {}
    for name, shape, dt in (
        ("xT", [D, S], F16), ("wq", [D, CH], F16), ("wk", [D, CH], F16),
        ("wv", [D, HPC * VW], F16), ("bq", [P, 2], F32), ("bk", [P, 2], F32),
        ("bv", [1, HPC * VW], F16), ("wp", [CH, D], F16),
        ("tri", [P, P], F16), ("ones1", [1, P], F16),
    ):
        ap[name] = nc.dram_tensor(name, shape, dt, kind="ExternalInput").ap()
    ap["out"] = nc.dram_tensor("out", [S, D], F32, kind="ExternalOutput").ap()

    with tile.TileContext(nc) as tc:
        emit_kernel(nc, tc, ap)
    nc.compile()
    return nc


def make_core_inputs(hidden_states, w_attn, b_attn, w_proj):
    """Host-side sharding: per-core input dicts (core = batch*4 + head_group)."""
    f16, f32 = np.float16, np.float32
    x = np.asarray(hidden_states, f32)
    w_attn = np.asarray(w_attn, f32)
    b_attn = np.asarray(b_attn, f32)
    w_proj = np.asarray(w_proj, f32)

    tri = (np.arange(P)[:, None] <= np.arange(P)[None, :]).astype(f16)
    ones_row = np.ones((1, P), f16)
    xTs = [np.ascontiguousarray(x[b].T).astype(f16) for b in range(B)]

    in_maps = []
    for core in range(NCORES):
        b, g = core // HPC, core % HPC
        wq = np.ascontiguousarray(w_attn[:, g * CH:(g + 1) * CH]).astype(f16)
        wk = np.ascontiguousarray(
            w_attn[:, D + g * CH:D + (g + 1) * CH]).astype(f16)
        wv = np.zeros((D, HPC * VW), f16)
        bv = np.zeros((1, HPC * VW), f16)
        for h in range(HPC):
            src = 2 * D + (g * HPC + h) * HD
            wv[:, h * VW:h * VW + HD] = w_attn[:, src:src + HD]
            bv[0, h * VW:h * VW + HD] = b_attn[src:src + HD]
            bv[0, h * VW + HD] = 1.0
        bq = np.ascontiguousarray(
            b_attn[g * CH:(g + 1) * CH].reshape(2, P).T)
        bk = np.ascontiguousarray(
            b_attn[D + g * CH:D + (g + 1) * CH].reshape(2, P).T)
        wp = np.ascontiguousarray(w_proj[g * CH:(g + 1) * CH, :]).astype(f16)
        in_maps.append({
            "xT": xTs[b], "wq": wq, "wk": wk, "wv": wv,
            "bq": bq, "bk": bk, "bv": bv, "wp": wp, "tri": tri,
            "ones1": ones_row,
        })
    return in_maps


_PROGRAM = None


def kernel(hidden_states, w_attn, b_attn, w_proj, b_proj):
    global _PROGRAM
    if _PROGRAM is None:
        _PROGRAM = build_program()
    in_maps = make_core_inputs(hidden_states, w_attn, b_attn, w_proj)
    res = run_bass_kernel_spmd(_PROGRAM, in_maps, core_ids=list(range(NCORES)))
    out = np.zeros((B, S, D), np.float32)
    for core in range(NCORES):
        out[core // HPC] += res.results[core]["out"]
    out += np.asarray(b_proj, np.float32)
    return out
